# revision 6
# baseline (speedup 1.0000x reference)
"""CenterLoss TRN2 Bass kernel, 8-way sharded.

Problem: x (8192, 256) f32, labels (8192,) i64, centers (8192, 256) f32.
Returns (loss_cent, loss_dis3, distmat3) like the reference.

Sharding: rows of x and rows of centers are split 1024/core (data parallel);
centersT is replicated as the matmul RHS. Per core, the RHS columns are
ROTATED by -1024*m so that the distmat3 diagonal falls at local column == local
row for every core; this makes diagonal masking compile-time static in the
SPMD program. Host un-rotates the output columns.

Device per core:
  part A (x vs centers):  e = 2*x@c^T - |x|^2 - |c|^2 accumulated in PSUM via
    two K=128 f32r matmuls plus a K=2 "fold" matmul carrying the sq terms;
    row-max of e (= -min dist) reduced per 512-col chunk straight from PSUM.
  part B (centers vs centers): same matmul structure; each chunk is drained by
    one DVE tensor_scalar (d = -e) with a fused running column-min accumulator;
    the diagonal chunk's min is recomputed with an affine_select mask; argmin
    is recovered with max_index (exact value match against the row min);
    row sums are computed analytically on the PE: sum_j d[i,j] =
    C*|c_i|^2 + sum_j |c_j|^2 - 2*c_i . sum_j c_j.
Host: gathers shards, un-rotates distmat3, sets the diagonal to the clip
floor, fixes the rare rows whose own center is the closest (exact recompute),
and reduces the final scalars.
"""
import os
import numpy as np

B = 8192
C = 8192
D = 256
NCORES = 8
R = B // NCORES          # rows per core
S = R // 128             # strips per core
CH = 512                 # chunk width
NCH = C // CH            # chunks per strip
CLAMP_MIN = 1e-12
BIG = 1.0e30

_cache = {}


def _build():
    import concourse.bass as bass
    import concourse.tile as tile
    from concourse import bacc, mybir
    from contextlib import ExitStack

    F32 = mybir.dt.float32
    F32R = mybir.dt.float32r
    U32 = mybir.dt.uint32
    Alu = mybir.AluOpType
    AX = mybir.AxisListType.X

    nc = bacc.Bacc("TRN2", target_bir_lowering=False, debug=False,
                   num_devices=NCORES)

    ctr_d = nc.dram_tensor("ctr", [D, C], F32, kind="ExternalInput").ap()
    xt2_d = nc.dram_tensor("xt2", [D, R], F32, kind="ExternalInput").ap()
    ct2_d = nc.dram_tensor("ct2", [D, R], F32, kind="ExternalInput").ap()
    folda_d = nc.dram_tensor("folda", [2, R], F32, kind="ExternalInput").ap()
    foldb_d = nc.dram_tensor("foldb", [2, R], F32, kind="ExternalInput").ap()
    rhs2_d = nc.dram_tensor("rhs2", [2, C], F32, kind="ExternalInput").ap()
    negs_d = nc.dram_tensor("negs", [D, 8], F32, kind="ExternalInput").ap()
    rhsb2_d = nc.dram_tensor("rhsb2", [2, 8], F32, kind="ExternalInput").ap()
    xo_d = nc.dram_tensor("xo", [128, S * D], F32, kind="ExternalInput").ap()
    cp_d = nc.dram_tensor("cp", [128, S * D], F32, kind="ExternalInput").ap()

    dist_d = nc.dram_tensor("dist", [R, C], F32, kind="ExternalOutput").ap()
    mina_d = nc.dram_tensor("mina", [128, S], F32, kind="ExternalOutput").ap()
    dpos_d = nc.dram_tensor("dpos", [128, S], F32, kind="ExternalOutput").ap()
    ridx_d = nc.dram_tensor("ridx", [128, S], U32, kind="ExternalOutput").ap()
    rsum_d = nc.dram_tensor("rsum", [128, S], F32, kind="ExternalOutput").ap()

    with tile.TileContext(nc) as tc, ExitStack() as ctx:
        resid = ctx.enter_context(tc.tile_pool(name="resid", bufs=1))
        bigp = ctx.enter_context(tc.tile_pool(name="bigp", bufs=2))
        smst = ctx.enter_context(tc.tile_pool(name="smst", bufs=1))
        work = ctx.enter_context(tc.tile_pool(name="work", bufs=2))
        ps = ctx.enter_context(tc.tile_pool(name="ps", bufs=6, space="PSUM"))
        psr = ctx.enter_context(tc.tile_pool(name="psr", bufs=2, space="PSUM"))

        # ---- load + round inputs to f32r ----
        def round_big(dram_half):
            stg = bigp.tile([128, C], F32, tag="bigstage")
            nc.sync.dma_start(stg[:], dram_half)
            out = resid.tile([128, C], F32R, tag=f"ct{id(dram_half)}")
            nc.scalar.copy(out[:], stg[:])
            return out

        ct0 = round_big(ctr_d[0:128, :])
        ct1 = round_big(ctr_d[128:256, :])

        def round_small(dram_ap, p, w, eng="vector"):
            stg = smst.tile([p, w], F32, tag="smstage")
            nc.sync.dma_start(stg[:], dram_ap)
            out = resid.tile([p, w], F32R, tag=f"sm{id(dram_ap)}")
            nc.vector.tensor_copy(out[:], stg[:])
            return out

        xt20 = round_small(xt2_d[0:128, :], 128, R)
        xt21 = round_small(xt2_d[128:256, :], 128, R)
        ct20 = round_small(ct2_d[0:128, :], 128, R)
        ct21 = round_small(ct2_d[128:256, :], 128, R)
        folda = round_small(folda_d[:], 2, R)
        foldb = round_small(foldb_d[:], 2, R)
        negs0 = round_small(negs_d[0:128, :], 128, 8)
        negs1 = round_small(negs_d[128:256, :], 128, 8)
        rhsb2 = round_small(rhsb2_d[:], 2, 8)

        rhs2stg = bigp.tile([2, C], F32, tag="bigstage")
        nc.sync.dma_start(rhs2stg[:], rhs2_d[:])
        rhs2 = resid.tile([2, C], F32R, tag="rhs2")
        nc.vector.tensor_copy(rhs2[:], rhs2stg[:])

        # ---- persistent result tables ----
        minaT = resid.tile([128, S], F32, tag="minaT")
        dposT = resid.tile([128, S], F32, tag="dposT")
        ridxT = resid.tile([128, S], U32, tag="ridxT")
        rsumT = resid.tile([128, S], F32, tag="rsumT")

        # ---- d_pos: dots of x rows with their own centers ----
        sqx8 = work.tile([128, S], F32, tag="dp8")
        sqc8 = work.tile([128, S], F32, tag="dp8b")
        dot8 = work.tile([128, S], F32, tag="dp8c")
        for t in range(S):
            blk = slice(t * D, (t + 1) * D)
            xob = work.tile([128, D], F32, tag="dpx")
            cpb = work.tile([128, D], F32, tag="dpc")
            nc.sync.dma_start(xob[:], xo_d[:, blk])
            nc.sync.dma_start(cpb[:], cp_d[:, blk])
            scr = work.tile([128, D], F32, tag="dpscr")
            nc.vector.scalar_tensor_tensor(
                out=scr[:], in0=xob[:], scalar=1.0, in1=xob[:],
                op0=Alu.mult, op1=Alu.mult, accum_out=sqx8[:, t:t + 1])
            scr = work.tile([128, D], F32, tag="dpscr")
            nc.vector.scalar_tensor_tensor(
                out=scr[:], in0=cpb[:], scalar=1.0, in1=cpb[:],
                op0=Alu.mult, op1=Alu.mult, accum_out=sqc8[:, t:t + 1])
            scr = work.tile([128, D], F32, tag="dpscr")
            nc.vector.scalar_tensor_tensor(
                out=scr[:], in0=xob[:], scalar=-2.0, in1=cpb[:],
                op0=Alu.mult, op1=Alu.mult, accum_out=dot8[:, t:t + 1])
        tmp8 = work.tile([128, S], F32, tag="dp8d")
        nc.vector.tensor_tensor(out=tmp8[:], in0=sqx8[:], in1=sqc8[:], op=Alu.add)
        nc.vector.tensor_tensor(out=dposT[:], in0=tmp8[:], in1=dot8[:], op=Alu.add)

        # ---- part A: min over all centers of ||x_i - c_j||^2 ----
        for s in range(S):
            ms = slice(s * 128, (s + 1) * 128)
            emax = work.tile([128, NCH], F32, tag="emaxA")
            for c in range(NCH):
                cs = slice(c * CH, (c + 1) * CH)
                pt = ps.tile([128, CH], F32, tag="psmm")
                nc.tensor.matmul(pt[:], xt20[:, ms], ct0[:, cs], start=True, stop=False)
                nc.tensor.matmul(pt[:], xt21[:, ms], ct1[:, cs], start=False, stop=False)
                nc.tensor.matmul(pt[:], folda[:, ms], rhs2[:, cs], start=False, stop=True)
                nc.vector.tensor_reduce(emax[:, c:c + 1], pt[:], axis=AX, op=Alu.max)
            smax = work.tile([128, 1], F32, tag="smaxA")
            nc.vector.tensor_reduce(smax[:], emax[:], axis=AX, op=Alu.max)
            nc.vector.tensor_scalar_mul(minaT[:, s:s + 1], smax[:], -1.0)

        # ---- part B: distmat3 strip, argmin, analytic rowsum ----
        for s in range(S):
            ms = slice(s * 128, (s + 1) * 128)
            cd = s // 4                      # chunk holding the diagonal block
            od = (s % 4) * 128               # diag block offset within chunk
            dstrip = bigp.tile([128, C], F32, tag="bigstage")
            dmin = work.tile([128, NCH], F32, tag="dminB")
            for c in range(NCH):
                cs = slice(c * CH, (c + 1) * CH)
                pt = ps.tile([128, CH], F32, tag="psmm")
                nc.tensor.matmul(pt[:], ct20[:, ms], ct0[:, cs], start=True, stop=False)
                nc.tensor.matmul(pt[:], ct21[:, ms], ct1[:, cs], start=False, stop=False)
                nc.tensor.matmul(pt[:], foldb[:, ms], rhs2[:, cs], start=False, stop=True)
                nc.vector.tensor_scalar(
                    dstrip[:, cs], pt[:], -1.0, None,
                    op0=Alu.mult, op1=Alu.min, accum_out=dmin[:, c:c + 1])

            # analytic rowsum on the PE
            pr = psr.tile([128, 8], F32, tag="psrB")
            nc.tensor.matmul(pr[:], ct20[:, ms], negs0[:], start=True, stop=False)
            nc.tensor.matmul(pr[:], ct21[:, ms], negs1[:], start=False, stop=False)
            nc.tensor.matmul(pr[:], foldb[:, ms], rhsb2[:], start=False, stop=True)
            nc.vector.tensor_copy(rsumT[:, s:s + 1], pr[:, 0:1])

            # redo the diagonal chunk's min with the diagonal masked out
            scrd = work.tile([128, 128], F32, tag="scrd")
            nc.gpsimd.affine_select(
                scrd[:], dstrip[:, s * 128:s * 128 + 128], pattern=[[1, 128]],
                compare_op=Alu.not_equal, fill=BIG, base=0, channel_multiplier=-1)
            cur = work.tile([128, 1], F32, tag="rmin0")
            nc.vector.tensor_reduce(cur[:], scrd[:], axis=AX, op=Alu.min)
            if od > 0:
                r2 = work.tile([128, 1], F32, tag="rmin1")
                nc.vector.tensor_reduce(
                    r2[:], dstrip[:, cd * CH:cd * CH + od], axis=AX, op=Alu.min)
                nxt = work.tile([128, 1], F32, tag="rmin2")
                nc.vector.tensor_tensor(out=nxt[:], in0=cur[:], in1=r2[:], op=Alu.min)
                cur = nxt
            if 384 - od > 0:
                r3 = work.tile([128, 1], F32, tag="rmin3")
                nc.vector.tensor_reduce(
                    r3[:], dstrip[:, s * 128 + 128:(cd + 1) * CH], axis=AX, op=Alu.min)
                nxt = work.tile([128, 1], F32, tag="rmin4")
                nc.vector.tensor_tensor(out=nxt[:], in0=cur[:], in1=r3[:], op=Alu.min)
                cur = nxt
            nc.vector.tensor_copy(dmin[:, cd:cd + 1], cur[:])

            rowmin = work.tile([128, 1], F32, tag="rowminB")
            nc.vector.tensor_reduce(rowmin[:], dmin[:], axis=AX, op=Alu.min)
            idx8 = work.tile([128, 8], U32, tag="idx8")
            nc.vector.max_index(idx8[:], rowmin[:].broadcast_to([128, 8]), dstrip[:])
            nc.vector.tensor_copy(ridxT[:, s:s + 1], idx8[:, 0:1])

            nc.sync.dma_start(dist_d[ms, :], dstrip[:])

        nc.sync.dma_start(mina_d[:], minaT[:])
        nc.sync.dma_start(dpos_d[:], dposT[:])
        nc.sync.dma_start(ridx_d[:], ridxT[:])
        nc.sync.dma_start(rsum_d[:], rsumT[:])

    nc.compile()
    return nc


def _get_nc():
    if "nc" not in _cache:
        _cache["nc"] = _build()
    return _cache["nc"]


def kernel(x, labels, centers):
    from concourse.bass_utils import run_bass_kernel_spmd

    x = np.asarray(x, dtype=np.float32)
    centers = np.asarray(centers, dtype=np.float32)
    labels_np = np.asarray(labels).astype(np.int64)

    nc = _get_nc()

    centersT = np.ascontiguousarray(centers.T)           # (256, 8192)
    sq_c = np.einsum("ij,ij->i", centers, centers).astype(np.float32)
    sq_x = np.einsum("ij,ij->i", x, x).astype(np.float32)
    Svec = centers.sum(0).astype(np.float32)             # (256,)
    sum_sq = np.float32(sq_c.astype(np.float64).sum())

    in_maps = []
    for m in range(NCORES):
        rows = slice(m * R, (m + 1) * R)
        ctr_m = np.roll(centersT, -R * m, axis=1)
        sqc_m = np.roll(sq_c, -R * m)
        xo_m = x[rows].reshape(S, 128, D).transpose(1, 0, 2).reshape(128, S * D)
        cp_full = centers[labels_np[rows]]
        cp_m = cp_full.reshape(S, 128, D).transpose(1, 0, 2).reshape(128, S * D)
        in_maps.append({
            "ctr": np.ascontiguousarray(ctr_m),
            "xt2": np.ascontiguousarray((2.0 * x[rows]).T),
            "ct2": np.ascontiguousarray((2.0 * centers[rows]).T),
            "folda": np.stack([-np.ones(R, np.float32), -sq_x[rows]]),
            "foldb": np.stack([-np.ones(R, np.float32), -sq_c[rows]]),
            "rhs2": np.stack([sqc_m, np.ones(C, np.float32)]),
            "negs": np.concatenate([(-Svec).reshape(D, 1), np.zeros((D, 7), np.float32)], axis=1),
            "rhsb2": np.concatenate([np.array([[-sum_sq], [-np.float32(C)]], np.float32), np.zeros((2, 7), np.float32)], axis=1),
            "xo": np.ascontiguousarray(xo_m),
            "cp": np.ascontiguousarray(cp_m),
        })

    trace = bool(int(os.environ.get("CK_TRACE", "0")))
    bres = run_bass_kernel_spmd(nc, in_maps, list(range(NCORES)), trace=trace)
    res = bres.results
    _cache["last_bres"] = bres

    # ---- host assembly ----
    dist = np.empty((C, C), np.float32)
    mina = np.empty(B, np.float32)
    dpos = np.empty(B, np.float32)
    rsum = np.empty(C, np.float64)
    mind = np.empty(C, np.int64)
    for m in range(NCORES):
        r = res[m]
        rows = slice(m * R, (m + 1) * R)
        dist[rows] = np.roll(r["dist"], R * m, axis=1)
        mina[rows] = r["mina"].T.reshape(-1)
        dpos[rows] = r["dpos"].T.reshape(-1)
        rsum[rows] = r["rsum"].T.reshape(-1).astype(np.float64)
        mind[rows] = (r["ridx"].T.reshape(-1).astype(np.int64) + R * m) % C

    np.fill_diagonal(dist, np.float32(CLAMP_MIN))

    # loss_cent: d_neg = min over j != label; fix rows where the own center
    # might be the overall argmin (or a near-tie) by exact recompute.
    dneg = mina.astype(np.float64)
    suspect = np.nonzero(mina >= dpos - 1e-2)[0]
    if suspect.size:
        xs = x[suspect].astype(np.float64)
        drow = (xs * xs).sum(1)[:, None] + (centers.astype(np.float64) ** 2).sum(1)[None, :] \
            - 2.0 * xs @ centers.astype(np.float64).T
        drow = np.clip(drow, CLAMP_MIN, 1e12)
        drow[np.arange(suspect.size), labels_np[suspect]] = np.inf
        dneg[suspect] = drow.min(1)
    loss_cent = np.float32((dpos.astype(np.float64) / (dneg + 1.0)).mean())

    loss_dis3 = np.float32(-rsum[mind].sum() / (C * C))

    return (np.float32(loss_cent), np.float32(loss_dis3), dist)
